# revision 1
# baseline (speedup 1.0000x reference)
"""Distributed paged GQA decode attention for Trainium2 (8 NeuronCores).

Strategy
--------
The 256 independent (batch, kv_head) pairs are the unit of work.  For each
pair the output depends only on the first seq_len+1 tokens of its paged
context, so the host gathers exactly the valid tokens from the paged cache
(emulating the decode_store_kv scatter first), pre-transposes K, folds the
softmax scale into q, casts everything to bf16, and ships per-core blobs.

Pairs are sorted by length and dealt into 32 groups of 8; group j becomes
"slot j" on every core (core c takes rank c of group j).  All cores share
one chunk count C_j = ceil(maxlen(group j)/128), which keeps the single
SPMD instruction stream identical across cores while wasting only ~10% in
padding.

Per core the device kernel holds everything in SBUF and, per slot j:
  scoresT[l,g] = KT_chunk.T @ qT          (PE, chunked by 128 tokens)
  e = exp(scoresT)                        (ACT, psum -> sbuf, bf16)
  o_unnorm/denom = e.T @ [V | 1]          (PE, accumulated in psum)
softmax max-subtraction is unnecessary (|score| <= ~7 for this regime) and
cancels between numerator and denominator; padded tokens contribute zero
because their V rows AND the ones-column are zeroed.  The final division
happens on the host during the unshard.
"""

import sys

sys.path.insert(0, "/opt/trn_rl_repo")

import numpy as np
import ml_dtypes

B = 32
H = 32
HKV = 8
D = 128
P = 16
G = H // HKV          # 4 query heads per kv head
SCALE = 0.08838834764831845
N_CORES = 8
CHUNK = 128
N_SLOTS = (B * HKV) // N_CORES   # 32 slots per core

BF16 = ml_dtypes.bfloat16

_GRAPH_CACHE = {}


def _build_graph(C):
    """Build the SPMD Bass graph for per-slot chunk counts C (len 32)."""
    from concourse import bacc, tile, mybir, bass

    NCH = sum(C)
    T = NCH * CHUNK

    nc = bacc.Bacc("TRN2", target_bir_lowering=False, debug=False,
                   num_devices=N_CORES)
    kt_d = nc.dram_tensor("kt", [128, T], mybir.dt.bfloat16,
                          kind="ExternalInput")
    vd_d = nc.dram_tensor("vd", [128, NCH * 129], mybir.dt.bfloat16,
                          kind="ExternalInput")
    qt_d = nc.dram_tensor("qt", [128, N_SLOTS * G], mybir.dt.bfloat16,
                          kind="ExternalInput")
    out_d = nc.dram_tensor("out", [N_SLOTS * G, 129], mybir.dt.float32,
                           kind="ExternalOutput")

    # group slots into DMA batches of ~4 slots (~0.9 MB per transfer)
    GROUP = 4
    groups = [list(range(g, min(g + GROUP, N_SLOTS)))
              for g in range(0, N_SLOTS, GROUP)]
    tok_off = np.cumsum([0] + [c * CHUNK for c in C]).tolist()   # per-slot
    ch_off = np.cumsum([0] + list(C)).tolist()

    with tile.TileContext(nc) as tc:
        with (
            tc.tile_pool(name="data", bufs=1) as data_pool,
            tc.tile_pool(name="work", bufs=1) as work_pool,
            tc.tile_pool(name="psum", bufs=1, space=bass.MemorySpace.PSUM)
                as psum_pool,
        ):
            qt = data_pool.tile([128, N_SLOTS * G], mybir.dt.bfloat16,
                                tag="qt", name="qt")
            nc.sync.dma_start(out=qt[:], in_=qt_d.ap()[:, :])

            kt_tiles = {}
            vd_tiles = {}
            for gi, slots in enumerate(groups):
                a_tok, b_tok = tok_off[slots[0]], tok_off[slots[-1] + 1]
                a_ch, b_ch = ch_off[slots[0]], ch_off[slots[-1] + 1]
                kt_g = data_pool.tile([128, b_tok - a_tok],
                                      mybir.dt.bfloat16,
                                      tag=f"kt{gi}", name=f"kt{gi}")
                nc.sync.dma_start(out=kt_g[:], in_=kt_d.ap()[:, a_tok:b_tok])
                vd_g = data_pool.tile([128, (b_ch - a_ch) * 129],
                                      mybir.dt.bfloat16,
                                      tag=f"vd{gi}", name=f"vd{gi}")
                nc.sync.dma_start(out=vd_g[:],
                                  in_=vd_d.ap()[:, a_ch * 129:b_ch * 129])
                for j in slots:
                    kt_tiles[j] = (kt_g, tok_off[j] - a_tok)
                    vd_tiles[j] = (vd_g, ch_off[j] - a_ch)

            def emit_pv(j, e):
                vd_g, coff = vd_tiles[j]
                po = psum_pool.tile([G, 129], mybir.dt.float32,
                                    tag="po", bufs=3, name=f"po{j}")
                for c in range(C[j]):
                    nc.tensor.matmul(
                        po[:, :],
                        e[:, G * c:G * (c + 1)],
                        vd_g[:, (coff + c) * 129:(coff + c + 1) * 129],
                        start=(c == 0), stop=(c == C[j] - 1),
                    )
                ob = work_pool.tile([G, 129], mybir.dt.float32,
                                    tag="ob", bufs=3, name=f"ob{j}")
                nc.vector.tensor_copy(ob[:], po[:])
                nc.scalar.dma_start(out=out_d.ap()[G * j:G * (j + 1), :],
                                    in_=ob[:])

            prev = None
            for j in range(N_SLOTS):
                kt_g, toff = kt_tiles[j]
                scores = psum_pool.tile([128, G * C[j]], mybir.dt.float32,
                                        tag="scores", bufs=3,
                                        name=f"scores{j}")
                for c in range(C[j]):
                    nc.tensor.matmul(
                        scores[:, G * c:G * (c + 1)],
                        kt_g[:, toff + CHUNK * c:toff + CHUNK * (c + 1)],
                        qt[:, G * j:G * (j + 1)],
                        start=True, stop=True,
                    )
                e = work_pool.tile([128, G * C[j]], mybir.dt.bfloat16,
                                   tag="e", bufs=3, name=f"e{j}")
                nc.scalar.activation(e[:], scores[:],
                                     mybir.ActivationFunctionType.Exp)
                if prev is not None:
                    emit_pv(*prev)
                prev = (j, e)
            emit_pv(*prev)

    nc.compile()
    return nc


def _prepare(q, k, v, k_cache, v_cache, bh_seq_lens, page_table,
             batch_mapping):
    """Host-side shard planning + gather.  Returns (C, in_maps, pair_map)."""
    q = np.asarray(q, dtype=np.float32)
    k = np.asarray(k, dtype=np.float32)
    v = np.asarray(v, dtype=np.float32)
    kcf = np.asarray(k_cache, dtype=np.float32).reshape(-1, D).copy()
    vcf = np.asarray(v_cache, dtype=np.float32).reshape(-1, D).copy()
    sl = np.asarray(bh_seq_lens)
    pt = np.asarray(page_table)
    bm = np.asarray(batch_mapping)

    seq = sl[bm]                      # [B, HKV]
    ptb = pt[bm].astype(np.int64)     # [B, HKV, M]

    # decode_store_kv: scatter new token into cache copies
    page_of = np.take_along_axis(ptb, (seq // P)[..., None].astype(np.int64),
                                 axis=-1)[..., 0]
    flat = page_of * P + (seq % P)
    kcf[flat.reshape(-1)] = k.reshape(-1, D)
    vcf[flat.reshape(-1)] = v.reshape(-1, D)

    lens = (seq + 1).reshape(-1)               # [256] valid tokens per pair
    order = np.argsort(-lens, kind="stable")   # longest first
    # group j = pairs order[8j..8j+8); core c <- rank c
    C = []
    for j in range(N_SLOTS):
        grp = order[N_CORES * j:N_CORES * (j + 1)]
        C.append(int(np.ceil(lens[grp].max() / CHUNK)))
    NCH = sum(C)
    T = NCH * CHUNK
    ch_off = np.cumsum([0] + list(C))
    tok_off = ch_off * CHUNK

    in_maps = []
    pair_map = []  # per core: list of (b, h) per slot
    for c in range(N_CORES):
        KT = np.zeros((128, T), dtype=BF16)
        V3 = np.zeros((NCH * 128, 129), dtype=BF16)
        qT = np.zeros((128, N_SLOTS * G), dtype=BF16)
        pm = []
        for j in range(N_SLOTS):
            pair = int(order[N_CORES * j + c])
            b, h = pair // HKV, pair % HKV
            pm.append((b, h))
            L = int(lens[pair])
            npages = (L + P - 1) // P
            tok = (ptb[b, h, :npages, None] * P
                   + np.arange(P, dtype=np.int64)).reshape(-1)[:L]
            Kg = kcf[tok]                       # [L, D] f32
            Vg = vcf[tok]
            KT[:, tok_off[j]:tok_off[j] + L] = Kg.T.astype(BF16)
            r0 = ch_off[j] * 128
            V3[r0:r0 + L, :D] = Vg.astype(BF16)
            V3[r0:r0 + L, D] = np.float32(1.0)
            qT[:, G * j:G * (j + 1)] = \
                (q[b, h * G:(h + 1) * G] * SCALE).T.astype(BF16)
        VD = np.ascontiguousarray(
            V3.reshape(NCH, 128, 129).transpose(1, 0, 2)
        ).reshape(128, NCH * 129)
        in_maps.append({"kt": KT, "vd": VD, "qt": qT})
        pair_map.append(pm)
    return tuple(C), in_maps, pair_map


def _run(inputs, trace=False, trace_cores=None):
    from concourse.bass_utils import run_bass_kernel_spmd

    C, in_maps, pair_map = _prepare(**inputs)
    if C not in _GRAPH_CACHE:
        _GRAPH_CACHE[C] = _build_graph(list(C))
    nc = _GRAPH_CACHE[C]

    res = run_bass_kernel_spmd(
        nc, in_maps, core_ids=list(range(N_CORES)),
        trace=trace, trace_cores=trace_cores,
    )

    out = np.zeros((B, H, D), dtype=np.float32)
    for c in range(N_CORES):
        oc = np.asarray(res.results[c]["out"], dtype=np.float32)  # [128,129]
        oc = oc.reshape(N_SLOTS, G, 129)
        for j, (b, h) in enumerate(pair_map[c]):
            out[b, h * G:(h + 1) * G] = oc[j, :, :D] / oc[j, :, D:D + 1]
    return out, res


def kernel(q, k, v, k_cache, v_cache, bh_seq_lens, page_table,
           batch_mapping):
    out, _ = _run(dict(q=q, k=k, v=v, k_cache=k_cache, v_cache=v_cache,
                       bh_seq_lens=bh_seq_lens, page_table=page_table,
                       batch_mapping=batch_mapping))
    return out


# revision 6
# speedup vs baseline: 1.0604x; 1.0604x over previous
"""Distributed paged GQA decode attention for Trainium2 (8 NeuronCores).

Strategy
--------
The 256 independent (batch, kv_head) pairs are the unit of work.  For each
pair the output depends only on the first seq_len+1 tokens of its paged
context, so the host gathers exactly the valid tokens from the paged cache
(emulating the decode_store_kv scatter first), pre-transposes K, folds the
softmax scale into q, casts everything to bf16, and ships per-core blobs.

Pairs are sorted by length and dealt into 32 groups of 8; group j becomes
"slot j" on every core (core c takes rank c of group j).  All cores share
one chunk count C_j = ceil(maxlen(group j)/128), which keeps the single
SPMD instruction stream identical across cores while wasting only ~10% in
padding.

Per core the device kernel holds everything in SBUF and, per slot j:
  scoresT[l,g] = KT_chunk.T @ qT          (PE, chunked by 128 tokens)
  e = exp(scoresT)                        (ACT, psum -> sbuf, bf16)
  o_unnorm/denom = e.T @ [V | 1]          (PE, accumulated in psum)
softmax max-subtraction is unnecessary (|score| <= ~7 for this regime) and
cancels between numerator and denominator; padded tokens contribute zero
because their V rows AND the ones-column are zeroed.  The final division
happens on the host during the unshard.
"""

import sys

sys.path.insert(0, "/opt/trn_rl_repo")

import numpy as np
import ml_dtypes

B = 32
H = 32
HKV = 8
D = 128
P = 16
G = H // HKV          # 4 query heads per kv head
SCALE = 0.08838834764831845
N_CORES = 8
CHUNK = 128
N_SLOTS = (B * HKV) // N_CORES   # 32 slots per core

BF16 = ml_dtypes.bfloat16

_GRAPH_CACHE = {}


def _build_graph(C):
    """Build the SPMD Bass graph for per-slot chunk counts C (len 32)."""
    from concourse import bacc, tile, mybir, bass

    NCH = sum(C)
    T = NCH * CHUNK

    nc = bacc.Bacc("TRN2", target_bir_lowering=False, debug=False,
                   num_devices=N_CORES)
    kt_d = nc.dram_tensor("kt", [128, T], mybir.dt.bfloat16,
                          kind="ExternalInput")
    vd_d = nc.dram_tensor("vd", [128, NCH * 129], mybir.dt.bfloat16,
                          kind="ExternalInput")
    qt_d = nc.dram_tensor("qt", [128, N_SLOTS * G], mybir.dt.bfloat16,
                          kind="ExternalInput")
    out_d = nc.dram_tensor("out", [G, N_SLOTS * 129], mybir.dt.float32,
                           kind="ExternalOutput")

    # group slots into DMA batches; small first groups so compute starts
    # early, bigger later ones to amortize per-DMA fixed cost
    sizes = [1, 1, 2, 4, 4, 4, 4, 4, 4, 4]
    assert sum(sizes) == N_SLOTS
    groups = []
    pos = 0
    for s in sizes:
        groups.append(list(range(pos, pos + s)))
        pos += s
    tok_off = np.cumsum([0] + [c * CHUNK for c in C]).tolist()   # per-slot
    ch_off = np.cumsum([0] + list(C)).tolist()

    with tile.TileContext(nc) as tc:
        with (
            tc.tile_pool(name="data", bufs=1) as data_pool,
            tc.tile_pool(name="work", bufs=1) as work_pool,
            tc.tile_pool(name="psum", bufs=1, space=bass.MemorySpace.PSUM)
                as psum_pool,
        ):
            qt = data_pool.tile([128, N_SLOTS * G], mybir.dt.bfloat16,
                                tag="qt", name="qt")
            nc.sync.dma_start(out=qt[:], in_=qt_d.ap()[:, :])

            kt_tiles = {}
            vd_tiles = {}
            for gi, slots in enumerate(groups):
                a_tok, b_tok = tok_off[slots[0]], tok_off[slots[-1] + 1]
                a_ch, b_ch = ch_off[slots[0]], ch_off[slots[-1] + 1]
                # K stream on the sync HWDGE ring, V stream on the gpsimd
                # SWDGE path: two independent descriptor queues feeding the
                # 16 SDMA engines, so the per-DMA fixed latencies overlap.
                kt_g = data_pool.tile([128, b_tok - a_tok],
                                      mybir.dt.bfloat16,
                                      tag=f"kt{gi}", name=f"kt{gi}")
                nc.sync.dma_start(out=kt_g[:], in_=kt_d.ap()[:, a_tok:b_tok])
                vd_g = data_pool.tile([128, (b_ch - a_ch) * 129],
                                      mybir.dt.bfloat16,
                                      tag=f"vd{gi}", name=f"vd{gi}")
                nc.gpsimd.dma_start(out=vd_g[:],
                                    in_=vd_d.ap()[:, a_ch * 129:b_ch * 129])
                for j in slots:
                    kt_tiles[j] = (kt_g, tok_off[j] - a_tok)
                    vd_tiles[j] = (vd_g, ch_off[j] - a_ch)

            # output staging: 4 tiles of 8 slots each; one gpsimd DMA per
            # stage once its 8 per-slot copies land
            OUT_GRP = 8
            stage_tiles = [
                work_pool.tile([G, OUT_GRP * 129], mybir.dt.float32,
                               tag=f"stage{s}", name=f"stage{s}")
                for s in range(N_SLOTS // OUT_GRP)
            ]

            def emit_pv(j, e):
                vd_g, coff = vd_tiles[j]
                po = psum_pool.tile([G, 129], mybir.dt.float32,
                                    tag="po", bufs=3, name=f"po{j}")
                for c in range(C[j]):
                    nc.tensor.matmul(
                        po[:, :],
                        e[:, G * c:G * (c + 1)],
                        vd_g[:, (coff + c) * 129:(coff + c + 1) * 129],
                        start=(c == 0), stop=(c == C[j] - 1),
                    )
                s, r = divmod(j, OUT_GRP)
                nc.vector.tensor_copy(
                    stage_tiles[s][:, r * 129:(r + 1) * 129], po[:])
                if r == OUT_GRP - 1:
                    nc.gpsimd.dma_start(
                        out=out_d.ap()[:, s * OUT_GRP * 129:
                                       (s + 1) * OUT_GRP * 129],
                        in_=stage_tiles[s][:],
                    )

            prev = None
            for j in range(N_SLOTS):
                kt_g, toff = kt_tiles[j]
                scores = psum_pool.tile([128, G * C[j]], mybir.dt.float32,
                                        tag="scores", bufs=3,
                                        name=f"scores{j}")
                for c in range(C[j]):
                    nc.tensor.matmul(
                        scores[:, G * c:G * (c + 1)],
                        kt_g[:, toff + CHUNK * c:toff + CHUNK * (c + 1)],
                        qt[:, G * j:G * (j + 1)],
                        start=True, stop=True,
                    )
                e = work_pool.tile([128, G * C[j]], mybir.dt.bfloat16,
                                   tag="e", bufs=3, name=f"e{j}")
                nc.scalar.activation(e[:], scores[:],
                                     mybir.ActivationFunctionType.Exp)
                if prev is not None:
                    emit_pv(*prev)
                prev = (j, e)
            emit_pv(*prev)

    nc.compile()
    return nc


def _prepare(q, k, v, k_cache, v_cache, bh_seq_lens, page_table,
             batch_mapping):
    """Host-side shard planning + gather.  Returns (C, in_maps, pair_map)."""
    q = np.asarray(q, dtype=np.float32)
    k = np.asarray(k, dtype=np.float32)
    v = np.asarray(v, dtype=np.float32)
    kcf = np.asarray(k_cache, dtype=np.float32).reshape(-1, D).copy()
    vcf = np.asarray(v_cache, dtype=np.float32).reshape(-1, D).copy()
    sl = np.asarray(bh_seq_lens)
    pt = np.asarray(page_table)
    bm = np.asarray(batch_mapping)

    seq = sl[bm]                      # [B, HKV]
    ptb = pt[bm].astype(np.int64)     # [B, HKV, M]

    # decode_store_kv: scatter new token into cache copies
    page_of = np.take_along_axis(ptb, (seq // P)[..., None].astype(np.int64),
                                 axis=-1)[..., 0]
    flat = page_of * P + (seq % P)
    kcf[flat.reshape(-1)] = k.reshape(-1, D)
    vcf[flat.reshape(-1)] = v.reshape(-1, D)

    lens = (seq + 1).reshape(-1)               # [256] valid tokens per pair
    order = np.argsort(-lens, kind="stable")   # longest first
    # group j = pairs order[8j..8j+8); core c <- rank c
    C = []
    for j in range(N_SLOTS):
        grp = order[N_CORES * j:N_CORES * (j + 1)]
        C.append(int(np.ceil(lens[grp].max() / CHUNK)))
    NCH = sum(C)
    T = NCH * CHUNK
    ch_off = np.cumsum([0] + list(C))
    tok_off = ch_off * CHUNK

    in_maps = []
    pair_map = []  # per core: list of (b, h) per slot
    for c in range(N_CORES):
        KT = np.zeros((128, T), dtype=BF16)
        V3 = np.zeros((NCH * 128, 129), dtype=BF16)
        qT = np.zeros((128, N_SLOTS * G), dtype=BF16)
        pm = []
        for j in range(N_SLOTS):
            pair = int(order[N_CORES * j + c])
            b, h = pair // HKV, pair % HKV
            pm.append((b, h))
            L = int(lens[pair])
            npages = (L + P - 1) // P
            tok = (ptb[b, h, :npages, None] * P
                   + np.arange(P, dtype=np.int64)).reshape(-1)[:L]
            Kg = kcf[tok]                       # [L, D] f32
            Vg = vcf[tok]
            KT[:, tok_off[j]:tok_off[j] + L] = Kg.T.astype(BF16)
            r0 = ch_off[j] * 128
            V3[r0:r0 + L, :D] = Vg.astype(BF16)
            V3[r0:r0 + L, D] = np.float32(1.0)
            qT[:, G * j:G * (j + 1)] = \
                (q[b, h * G:(h + 1) * G] * SCALE).T.astype(BF16)
        VD = np.ascontiguousarray(
            V3.reshape(NCH, 128, 129).transpose(1, 0, 2)
        ).reshape(128, NCH * 129)
        in_maps.append({"kt": KT, "vd": VD, "qt": qT})
        pair_map.append(pm)
    return tuple(C), in_maps, pair_map


def _run(inputs, trace=False, trace_cores=None):
    from concourse.bass_utils import run_bass_kernel_spmd

    C, in_maps, pair_map = _prepare(**inputs)
    if C not in _GRAPH_CACHE:
        _GRAPH_CACHE[C] = _build_graph(list(C))
    nc = _GRAPH_CACHE[C]

    res = run_bass_kernel_spmd(
        nc, in_maps, core_ids=list(range(N_CORES)),
        trace=trace, trace_cores=trace_cores,
    )

    out = np.zeros((B, H, D), dtype=np.float32)
    for c in range(N_CORES):
        oc = np.asarray(res.results[c]["out"], dtype=np.float32)
        oc = oc.reshape(G, N_SLOTS, 129).transpose(1, 0, 2)  # [slot, g, 129]
        for j, (b, h) in enumerate(pair_map[c]):
            out[b, h * G:(h + 1) * G] = oc[j, :, :D] / oc[j, :, D:D + 1]
    return out, res


def kernel(q, k, v, k_cache, v_cache, bh_seq_lens, page_table,
           batch_mapping):
    out, _ = _run(dict(q=q, k=k, v=v, k_cache=k_cache, v_cache=v_cache,
                       bh_seq_lens=bh_seq_lens, page_table=page_table,
                       batch_mapping=batch_mapping))
    return out


# revision 7
# speedup vs baseline: 1.2029x; 1.1344x over previous
"""Distributed paged GQA decode attention for Trainium2 (8 NeuronCores).

Strategy
--------
The 256 independent (batch, kv_head) pairs are the unit of work.  For each
pair the output depends only on the first seq_len+1 tokens of its paged
context, so the host gathers exactly the valid tokens from the paged cache
(emulating the decode_store_kv scatter first), pre-transposes K, folds the
softmax scale into q, casts everything to bf16, and ships per-core blobs.

Pairs are sorted by length and dealt into 32 groups of 8; group j becomes
"slot j" on every core (core c takes rank c of group j).  All cores share
one chunk count C_j = ceil(maxlen(group j)/128), which keeps the single
SPMD instruction stream identical across cores while wasting only ~10% in
padding.

Per core the device kernel holds everything in SBUF and, per slot j:
  scoresT[l,g] = KT_chunk.T @ qT          (PE, chunked by 128 tokens)
  e = exp(scoresT)                        (ACT, psum -> sbuf, bf16)
  o_unnorm/denom = e.T @ [V | 1]          (PE, accumulated in psum)
softmax max-subtraction is unnecessary (|score| <= ~7 for this regime) and
cancels between numerator and denominator; padded tokens contribute zero
because their V rows AND the ones-column are zeroed.  The final division
happens on the host during the unshard.
"""

import sys

sys.path.insert(0, "/opt/trn_rl_repo")

import numpy as np
import ml_dtypes

B = 32
H = 32
HKV = 8
D = 128
P = 16
G = H // HKV          # 4 query heads per kv head
SCALE = 0.08838834764831845
N_CORES = 8
CHUNK = 128
N_SLOTS = (B * HKV) // N_CORES   # 32 slots per core

BF16 = ml_dtypes.bfloat16

_GRAPH_CACHE = {}


def _build_graph(C):
    """Build the SPMD Bass graph for per-slot chunk counts C (len 32)."""
    from concourse import bacc, tile, mybir, bass

    NCH = sum(C)
    T = NCH * CHUNK

    nc = bacc.Bacc("TRN2", target_bir_lowering=False, debug=False,
                   num_devices=N_CORES)
    kt_d = nc.dram_tensor("kt", [128, T], mybir.dt.bfloat16,
                          kind="ExternalInput")
    vd_d = nc.dram_tensor("vd", [128, NCH * 129], mybir.dt.bfloat16,
                          kind="ExternalInput")
    qt_d = nc.dram_tensor("qt", [128, N_SLOTS * G], mybir.dt.bfloat16,
                          kind="ExternalInput")
    out_d = nc.dram_tensor("out", [G, N_SLOTS * 129], mybir.dt.float32,
                           kind="ExternalOutput")

    # group slots into DMA batches; small first groups so compute starts
    # early, bigger later ones to amortize per-DMA fixed cost
    sizes = [1, 1, 2, 4, 4, 4, 4, 4, 4, 4]
    assert sum(sizes) == N_SLOTS
    groups = []
    pos = 0
    for s in sizes:
        groups.append(list(range(pos, pos + s)))
        pos += s
    tok_off = np.cumsum([0] + [c * CHUNK for c in C]).tolist()   # per-slot
    ch_off = np.cumsum([0] + list(C)).tolist()

    with tile.TileContext(nc) as tc:
        with (
            tc.tile_pool(name="data", bufs=1) as data_pool,
            tc.tile_pool(name="work", bufs=1) as work_pool,
            tc.tile_pool(name="psum", bufs=1, space=bass.MemorySpace.PSUM)
                as psum_pool,
        ):
            qt = data_pool.tile([128, N_SLOTS * G], mybir.dt.bfloat16,
                                tag="qt", name="qt")
            nc.sync.dma_start(out=qt[:], in_=qt_d.ap()[:, :])

            kt_tiles = {}
            vd_tiles = {}
            for gi, slots in enumerate(groups):
                a_tok, b_tok = tok_off[slots[0]], tok_off[slots[-1] + 1]
                a_ch, b_ch = ch_off[slots[0]], ch_off[slots[-1] + 1]
                # everything on the single sync HWDGE ring: measured
                # ~424 GB/s sequential vs ~290 GB/s when two descriptor
                # queues interleave on the SDMA engines
                kt_g = data_pool.tile([128, b_tok - a_tok],
                                      mybir.dt.bfloat16,
                                      tag=f"kt{gi}", name=f"kt{gi}")
                nc.sync.dma_start(out=kt_g[:], in_=kt_d.ap()[:, a_tok:b_tok])
                vd_g = data_pool.tile([128, (b_ch - a_ch) * 129],
                                      mybir.dt.bfloat16,
                                      tag=f"vd{gi}", name=f"vd{gi}")
                nc.sync.dma_start(out=vd_g[:],
                                  in_=vd_d.ap()[:, a_ch * 129:b_ch * 129])
                for j in slots:
                    kt_tiles[j] = (kt_g, tok_off[j] - a_tok)
                    vd_tiles[j] = (vd_g, ch_off[j] - a_ch)

            # output staging: 4 tiles of 8 slots each; one gpsimd DMA per
            # stage once its 8 per-slot copies land
            OUT_GRP = 8
            stage_tiles = [
                work_pool.tile([G, OUT_GRP * 129], mybir.dt.float32,
                               tag=f"stage{s}", name=f"stage{s}")
                for s in range(N_SLOTS // OUT_GRP)
            ]

            def emit_pv(j, e):
                vd_g, coff = vd_tiles[j]
                po = psum_pool.tile([G, 129], mybir.dt.float32,
                                    tag="po", bufs=3, name=f"po{j}")
                for c in range(C[j]):
                    nc.tensor.matmul(
                        po[:, :],
                        e[:, G * c:G * (c + 1)],
                        vd_g[:, (coff + c) * 129:(coff + c + 1) * 129],
                        start=(c == 0), stop=(c == C[j] - 1),
                    )
                s, r = divmod(j, OUT_GRP)
                nc.vector.tensor_copy(
                    stage_tiles[s][:, r * 129:(r + 1) * 129], po[:])
                if r == OUT_GRP - 1:
                    nc.sync.dma_start(
                        out=out_d.ap()[:, s * OUT_GRP * 129:
                                       (s + 1) * OUT_GRP * 129],
                        in_=stage_tiles[s][:],
                    )

            prev = None
            for j in range(N_SLOTS):
                kt_g, toff = kt_tiles[j]
                scores = psum_pool.tile([128, G * C[j]], mybir.dt.float32,
                                        tag="scores", bufs=3,
                                        name=f"scores{j}")
                for c in range(C[j]):
                    nc.tensor.matmul(
                        scores[:, G * c:G * (c + 1)],
                        kt_g[:, toff + CHUNK * c:toff + CHUNK * (c + 1)],
                        qt[:, G * j:G * (j + 1)],
                        start=True, stop=True,
                    )
                e = work_pool.tile([128, G * C[j]], mybir.dt.bfloat16,
                                   tag="e", bufs=3, name=f"e{j}")
                nc.scalar.activation(e[:], scores[:],
                                     mybir.ActivationFunctionType.Exp)
                if prev is not None:
                    emit_pv(*prev)
                prev = (j, e)
            emit_pv(*prev)

    nc.compile()
    return nc


def _prepare(q, k, v, k_cache, v_cache, bh_seq_lens, page_table,
             batch_mapping):
    """Host-side shard planning + gather.  Returns (C, in_maps, pair_map)."""
    q = np.asarray(q, dtype=np.float32)
    k = np.asarray(k, dtype=np.float32)
    v = np.asarray(v, dtype=np.float32)
    kcf = np.asarray(k_cache, dtype=np.float32).reshape(-1, D).copy()
    vcf = np.asarray(v_cache, dtype=np.float32).reshape(-1, D).copy()
    sl = np.asarray(bh_seq_lens)
    pt = np.asarray(page_table)
    bm = np.asarray(batch_mapping)

    seq = sl[bm]                      # [B, HKV]
    ptb = pt[bm].astype(np.int64)     # [B, HKV, M]

    # decode_store_kv: scatter new token into cache copies
    page_of = np.take_along_axis(ptb, (seq // P)[..., None].astype(np.int64),
                                 axis=-1)[..., 0]
    flat = page_of * P + (seq % P)
    kcf[flat.reshape(-1)] = k.reshape(-1, D)
    vcf[flat.reshape(-1)] = v.reshape(-1, D)

    lens = (seq + 1).reshape(-1)               # [256] valid tokens per pair
    order = np.argsort(-lens, kind="stable")   # longest first
    # group j = pairs order[8j..8j+8); core c <- rank c
    C = []
    for j in range(N_SLOTS):
        grp = order[N_CORES * j:N_CORES * (j + 1)]
        C.append(int(np.ceil(lens[grp].max() / CHUNK)))
    NCH = sum(C)
    T = NCH * CHUNK
    ch_off = np.cumsum([0] + list(C))
    tok_off = ch_off * CHUNK

    in_maps = []
    pair_map = []  # per core: list of (b, h) per slot
    for c in range(N_CORES):
        KT = np.zeros((128, T), dtype=BF16)
        V3 = np.zeros((NCH * 128, 129), dtype=BF16)
        qT = np.zeros((128, N_SLOTS * G), dtype=BF16)
        pm = []
        for j in range(N_SLOTS):
            pair = int(order[N_CORES * j + c])
            b, h = pair // HKV, pair % HKV
            pm.append((b, h))
            L = int(lens[pair])
            npages = (L + P - 1) // P
            tok = (ptb[b, h, :npages, None] * P
                   + np.arange(P, dtype=np.int64)).reshape(-1)[:L]
            Kg = kcf[tok]                       # [L, D] f32
            Vg = vcf[tok]
            KT[:, tok_off[j]:tok_off[j] + L] = Kg.T.astype(BF16)
            r0 = ch_off[j] * 128
            V3[r0:r0 + L, :D] = Vg.astype(BF16)
            V3[r0:r0 + L, D] = np.float32(1.0)
            qT[:, G * j:G * (j + 1)] = \
                (q[b, h * G:(h + 1) * G] * SCALE).T.astype(BF16)
        VD = np.ascontiguousarray(
            V3.reshape(NCH, 128, 129).transpose(1, 0, 2)
        ).reshape(128, NCH * 129)
        in_maps.append({"kt": KT, "vd": VD, "qt": qT})
        pair_map.append(pm)
    return tuple(C), in_maps, pair_map


def _run(inputs, trace=False, trace_cores=None):
    from concourse.bass_utils import run_bass_kernel_spmd

    C, in_maps, pair_map = _prepare(**inputs)
    if C not in _GRAPH_CACHE:
        _GRAPH_CACHE[C] = _build_graph(list(C))
    nc = _GRAPH_CACHE[C]

    res = run_bass_kernel_spmd(
        nc, in_maps, core_ids=list(range(N_CORES)),
        trace=trace, trace_cores=trace_cores,
    )

    out = np.zeros((B, H, D), dtype=np.float32)
    for c in range(N_CORES):
        oc = np.asarray(res.results[c]["out"], dtype=np.float32)
        oc = oc.reshape(G, N_SLOTS, 129).transpose(1, 0, 2)  # [slot, g, 129]
        for j, (b, h) in enumerate(pair_map[c]):
            out[b, h * G:(h + 1) * G] = oc[j, :, :D] / oc[j, :, D:D + 1]
    return out, res


def kernel(q, k, v, k_cache, v_cache, bh_seq_lens, page_table,
           batch_mapping):
    out, _ = _run(dict(q=q, k=k, v=v, k_cache=k_cache, v_cache=v_cache,
                       bh_seq_lens=bh_seq_lens, page_table=page_table,
                       batch_mapping=batch_mapping))
    return out


# revision 8
# speedup vs baseline: 1.2032x; 1.0003x over previous
"""Distributed paged GQA decode attention for Trainium2 (8 NeuronCores).

Strategy
--------
The 256 independent (batch, kv_head) pairs are the unit of work.  For each
pair the output depends only on the first seq_len+1 tokens of its paged
context, so the host gathers exactly the valid tokens from the paged cache
(emulating the decode_store_kv scatter first), pre-transposes K, folds the
softmax scale into q, casts everything to bf16, and ships per-core blobs.

Pairs are sorted by length and dealt into 32 groups of 8; group j becomes
"slot j" on every core (core c takes rank c of group j).  All cores share
one chunk count C_j = ceil(maxlen(group j)/128), which keeps the single
SPMD instruction stream identical across cores while wasting only ~10% in
padding.

Per core the device kernel holds everything in SBUF and, per slot j:
  scoresT[l,g] = KT_chunk.T @ qT          (PE, chunked by 128 tokens)
  e = exp(scoresT)                        (ACT, psum -> sbuf, bf16)
  o_unnorm/denom = e.T @ [V | 1]          (PE, accumulated in psum)
softmax max-subtraction is unnecessary (|score| <= ~7 for this regime) and
cancels between numerator and denominator; padded tokens contribute zero
because their V rows AND the ones-column are zeroed.  The final division
happens on the host during the unshard.
"""

import sys

sys.path.insert(0, "/opt/trn_rl_repo")

import numpy as np
import ml_dtypes

B = 32
H = 32
HKV = 8
D = 128
P = 16
G = H // HKV          # 4 query heads per kv head
SCALE = 0.08838834764831845
N_CORES = 8
CHUNK = 128
N_SLOTS = (B * HKV) // N_CORES   # 32 slots per core

BF16 = ml_dtypes.bfloat16

_GRAPH_CACHE = {}


def _build_graph(C):
    """Build the SPMD Bass graph for per-slot chunk counts C (len 32)."""
    from concourse import bacc, tile, mybir, bass

    NCH = sum(C)
    T = NCH * CHUNK

    nc = bacc.Bacc("TRN2", target_bir_lowering=False, debug=False,
                   num_devices=N_CORES)
    kt_d = nc.dram_tensor("kt", [128, T], mybir.dt.bfloat16,
                          kind="ExternalInput")
    vd_d = nc.dram_tensor("vd", [128, NCH * 129], mybir.dt.bfloat16,
                          kind="ExternalInput")
    qt_d = nc.dram_tensor("qt", [128, N_SLOTS * G], mybir.dt.bfloat16,
                          kind="ExternalInput")
    out_d = nc.dram_tensor("out", [G, N_SLOTS * 129], mybir.dt.float32,
                           kind="ExternalOutput")

    # group slots into DMA batches; small first groups so compute starts
    # early, bigger later ones to amortize per-DMA fixed cost
    sizes = [1, 1, 2, 4, 4, 4, 4, 4, 4, 4]
    assert sum(sizes) == N_SLOTS
    groups = []
    pos = 0
    for s in sizes:
        groups.append(list(range(pos, pos + s)))
        pos += s
    tok_off = np.cumsum([0] + [c * CHUNK for c in C]).tolist()   # per-slot
    ch_off = np.cumsum([0] + list(C)).tolist()

    with tile.TileContext(nc) as tc:
        with (
            tc.tile_pool(name="data", bufs=1) as data_pool,
            tc.tile_pool(name="work", bufs=1) as work_pool,
            tc.tile_pool(name="psum", bufs=1, space=bass.MemorySpace.PSUM)
                as psum_pool,
        ):
            qt = data_pool.tile([128, N_SLOTS * G], mybir.dt.bfloat16,
                                tag="qt", name="qt")
            nc.sync.dma_start(out=qt[:], in_=qt_d.ap()[:, :])

            kt_tiles = {}
            vd_tiles = {}
            for gi, slots in enumerate(groups):
                a_tok, b_tok = tok_off[slots[0]], tok_off[slots[-1] + 1]
                a_ch, b_ch = ch_off[slots[0]], ch_off[slots[-1] + 1]
                # everything on the single sync HWDGE ring: measured
                # ~424 GB/s sequential vs ~290 GB/s when two descriptor
                # queues interleave on the SDMA engines
                kt_g = data_pool.tile([128, b_tok - a_tok],
                                      mybir.dt.bfloat16,
                                      tag=f"kt{gi}", name=f"kt{gi}")
                nc.sync.dma_start(out=kt_g[:], in_=kt_d.ap()[:, a_tok:b_tok])
                vd_g = data_pool.tile([128, (b_ch - a_ch) * 129],
                                      mybir.dt.bfloat16,
                                      tag=f"vd{gi}", name=f"vd{gi}")
                nc.sync.dma_start(out=vd_g[:],
                                  in_=vd_d.ap()[:, a_ch * 129:b_ch * 129])
                for j in slots:
                    kt_tiles[j] = (kt_g, tok_off[j] - a_tok)
                    vd_tiles[j] = (vd_g, ch_off[j] - a_ch)

            # output staging: 4 tiles of 8 slots each; one gpsimd DMA per
            # stage once its 8 per-slot copies land
            OUT_GRP = 8
            stage_tiles = [
                work_pool.tile([G, OUT_GRP * 129], mybir.dt.float32,
                               tag=f"stage{s}", name=f"stage{s}")
                for s in range(N_SLOTS // OUT_GRP)
            ]

            def emit_pv(j, e):
                vd_g, coff = vd_tiles[j]
                po = psum_pool.tile([G, 129], mybir.dt.float32,
                                    tag="po", bufs=3, name=f"po{j}")
                for c in range(C[j]):
                    nc.tensor.matmul(
                        po[:, :],
                        e[:, G * c:G * (c + 1)],
                        vd_g[:, (coff + c) * 129:(coff + c + 1) * 129],
                        start=(c == 0), stop=(c == C[j] - 1),
                    )
                s, r = divmod(j, OUT_GRP)
                nc.vector.tensor_copy(
                    stage_tiles[s][:, r * 129:(r + 1) * 129], po[:])
                if r == OUT_GRP - 1:
                    # SWDGE path: keeps the bulk-input HWDGE ring free of
                    # the ~1-2us HBM-write receipt stalls of output DMAs
                    nc.gpsimd.dma_start(
                        out=out_d.ap()[:, s * OUT_GRP * 129:
                                       (s + 1) * OUT_GRP * 129],
                        in_=stage_tiles[s][:],
                    )

            prev = None
            for j in range(N_SLOTS):
                kt_g, toff = kt_tiles[j]
                scores = psum_pool.tile([128, G * C[j]], mybir.dt.float32,
                                        tag="scores", bufs=3,
                                        name=f"scores{j}")
                for c in range(C[j]):
                    nc.tensor.matmul(
                        scores[:, G * c:G * (c + 1)],
                        kt_g[:, toff + CHUNK * c:toff + CHUNK * (c + 1)],
                        qt[:, G * j:G * (j + 1)],
                        start=True, stop=True,
                    )
                e = work_pool.tile([128, G * C[j]], mybir.dt.bfloat16,
                                   tag="e", bufs=3, name=f"e{j}")
                nc.scalar.activation(e[:], scores[:],
                                     mybir.ActivationFunctionType.Exp)
                if prev is not None:
                    emit_pv(*prev)
                prev = (j, e)
            emit_pv(*prev)

    nc.compile()
    return nc


def _prepare(q, k, v, k_cache, v_cache, bh_seq_lens, page_table,
             batch_mapping):
    """Host-side shard planning + gather.  Returns (C, in_maps, pair_map)."""
    q = np.asarray(q, dtype=np.float32)
    k = np.asarray(k, dtype=np.float32)
    v = np.asarray(v, dtype=np.float32)
    kcf = np.asarray(k_cache, dtype=np.float32).reshape(-1, D).copy()
    vcf = np.asarray(v_cache, dtype=np.float32).reshape(-1, D).copy()
    sl = np.asarray(bh_seq_lens)
    pt = np.asarray(page_table)
    bm = np.asarray(batch_mapping)

    seq = sl[bm]                      # [B, HKV]
    ptb = pt[bm].astype(np.int64)     # [B, HKV, M]

    # decode_store_kv: scatter new token into cache copies
    page_of = np.take_along_axis(ptb, (seq // P)[..., None].astype(np.int64),
                                 axis=-1)[..., 0]
    flat = page_of * P + (seq % P)
    kcf[flat.reshape(-1)] = k.reshape(-1, D)
    vcf[flat.reshape(-1)] = v.reshape(-1, D)

    lens = (seq + 1).reshape(-1)               # [256] valid tokens per pair
    order = np.argsort(-lens, kind="stable")   # longest first
    # group j = pairs order[8j..8j+8); core c <- rank c
    C = []
    for j in range(N_SLOTS):
        grp = order[N_CORES * j:N_CORES * (j + 1)]
        C.append(int(np.ceil(lens[grp].max() / CHUNK)))
    NCH = sum(C)
    T = NCH * CHUNK
    ch_off = np.cumsum([0] + list(C))
    tok_off = ch_off * CHUNK

    in_maps = []
    pair_map = []  # per core: list of (b, h) per slot
    for c in range(N_CORES):
        KT = np.zeros((128, T), dtype=BF16)
        V3 = np.zeros((NCH * 128, 129), dtype=BF16)
        qT = np.zeros((128, N_SLOTS * G), dtype=BF16)
        pm = []
        for j in range(N_SLOTS):
            pair = int(order[N_CORES * j + c])
            b, h = pair // HKV, pair % HKV
            pm.append((b, h))
            L = int(lens[pair])
            npages = (L + P - 1) // P
            tok = (ptb[b, h, :npages, None] * P
                   + np.arange(P, dtype=np.int64)).reshape(-1)[:L]
            Kg = kcf[tok]                       # [L, D] f32
            Vg = vcf[tok]
            KT[:, tok_off[j]:tok_off[j] + L] = Kg.T.astype(BF16)
            r0 = ch_off[j] * 128
            V3[r0:r0 + L, :D] = Vg.astype(BF16)
            V3[r0:r0 + L, D] = np.float32(1.0)
            qT[:, G * j:G * (j + 1)] = \
                (q[b, h * G:(h + 1) * G] * SCALE).T.astype(BF16)
        VD = np.ascontiguousarray(
            V3.reshape(NCH, 128, 129).transpose(1, 0, 2)
        ).reshape(128, NCH * 129)
        in_maps.append({"kt": KT, "vd": VD, "qt": qT})
        pair_map.append(pm)
    return tuple(C), in_maps, pair_map


def _run(inputs, trace=False, trace_cores=None):
    from concourse.bass_utils import run_bass_kernel_spmd

    C, in_maps, pair_map = _prepare(**inputs)
    if C not in _GRAPH_CACHE:
        _GRAPH_CACHE[C] = _build_graph(list(C))
    nc = _GRAPH_CACHE[C]

    res = run_bass_kernel_spmd(
        nc, in_maps, core_ids=list(range(N_CORES)),
        trace=trace, trace_cores=trace_cores,
    )

    out = np.zeros((B, H, D), dtype=np.float32)
    for c in range(N_CORES):
        oc = np.asarray(res.results[c]["out"], dtype=np.float32)
        oc = oc.reshape(G, N_SLOTS, 129).transpose(1, 0, 2)  # [slot, g, 129]
        for j, (b, h) in enumerate(pair_map[c]):
            out[b, h * G:(h + 1) * G] = oc[j, :, :D] / oc[j, :, D:D + 1]
    return out, res


def kernel(q, k, v, k_cache, v_cache, bh_seq_lens, page_table,
           batch_mapping):
    out, _ = _run(dict(q=q, k=k, v=v, k_cache=k_cache, v_cache=v_cache,
                       bh_seq_lens=bh_seq_lens, page_table=page_table,
                       batch_mapping=batch_mapping))
    return out
